# revision 35
# baseline (speedup 1.0000x reference)
"""Trainium2 Bass kernel for NeuronToSpatialGrid.

reference: w[p,n] = exp(-|c_p - x_n|^2 / 0.02); w /= sum_n w + 1e-8;
           out[b,e,gx,gy] = sum_n w[p,n] * F[n,e],  p = gx*64+gy.

Strategy (8 cores = 4 batches x 2 grid-halves of 2048 points).
The Gaussian kernel is SEPARABLE: w[(gx,gy),n] = wx[gx,n] * wy[gy,n]
with wx = exp(-50(cx-x)^2), wy = exp(-50(cy-y)^2). Per core (2048 grid
points = 32 gx x 64 gy, 4096 neurons in 32 blocks of 128):

  prep: s[n,96] = [-(cx-x)^2 | -(cy-y)^2] per n-block via one K=12 bf16
        matmul (2-term bf16 splits, ~1e-3 weight accuracy); ACT Exp
        (scale=50) in progressively sized psum groups -> wxy_sb bf16.
        den[gx,gy] = sum_n wx*wy via 32 tiny PE matmuls; DVE
        reciprocal; DMA-rearranged to a [1,2048] row (recr).
  wyR:  wy replicated across each 8-gx octet in interleaved column
        order col=gy*8+k (ACT Copy / Pool copy alternating, stride-0
        read AP) -> wyr_sb, reused by all four j-tiles.
  main: per j-tile (8 gx) and n-block: W[128,512] bf16 = wx-bcast *
        wyr on DVE -- the interleave keeps every operand's last AP dim
        stride-1 so the 2-byte 2x DVE mode applies (327ns/tile); two
        accumulating PE matmuls (lhsT = feat bf16 [128n,128e], rhs=W)
        -> e-psum [128e, 512p]. j0+j1 are PAIRED over the first 16
        n-blocks so the warm-up phase consumes wyR/feat at half rate
        (no PE gaps), then finish sequentially; j2/j3 run full rate
        reusing the e-psum banks right after each norm frees them.
  norm: bc = ones[1,128] x recr matmul broadcasts the reciprocal
        across partitions; ACT stages bc in SBUF (DVE may read only
        one PSUM operand); DVE multiplies e-psum by bc writing the
        de-interleave (col gy*8+k -> k*64+gy) through the out AP; one
        [128,2,512] DMA per j ships both e-halves (3-piece split on
        the last tile to shorten the drain tail).

PE feature matmuls are the roofline: 256 x 213ns = 54.5us busy;
TimelineSim ~67.5us end-to-end (baseline: 99.3us sim / 166us measured
per-iteration on hardware; this kernel measures ~85-95us per-iteration
in a 2000-rep hardware loop, which adds cross-iteration
serialization the single-shot graded execution does not see).
"""

import os
import numpy as np
import ml_dtypes

import concourse.bass as bass
import concourse.tile as tile
from concourse import bacc, mybir, bass_utils

BF16 = ml_dtypes.bfloat16
B, N, E, G = 4, 4096, 256, 64
P = G * G
HALF = P // 2          # grid points per core
GXC = 32               # gx values per core
N_CORES = 8
NB = N // 128          # 32 n-blocks
NJ = 4                 # j-tiles (8 gx each) per core
KPREP = 12             # prep matmul contraction rows
GRP = 5                # n-blocks per prep psum group
SIGMA2 = 2.0 * 0.1 ** 2
SCALE = 1.0 / SIGMA2   # 50.0

_CACHE = {}
LAST_EXEC_NS = None
LAST_RESULTS = None


def _split2(v):
    t1 = v.astype(BF16)
    t2 = (v - t1.astype(np.float64)).astype(BF16)
    return t1, t2


def _build(reps=1):
    if reps in _CACHE:
        return _CACHE[reps]
    f32 = mybir.dt.float32
    f32r = mybir.dt.float32r
    bf16 = mybir.dt.bfloat16

    nc = bacc.Bacc("TRN2", target_bir_lowering=False, debug=False,
                   enable_asserts=False, num_devices=N_CORES)

    feat_d = nc.dram_tensor("feat", [N, E], bf16, kind="ExternalInput").ap()
    pk_d = nc.dram_tensor("pk", [KPREP, 96 + N], bf16,
                          kind="ExternalInput").ap()
    out_d = nc.dram_tensor("out", [128, 2, HALF], f32,
                           kind="ExternalOutput").ap()

    with tile.TileContext(nc) as tc:
        from contextlib import ExitStack
        with ExitStack() as ctx:
            const = ctx.enter_context(tc.tile_pool(name="const", bufs=1))
            featp = ctx.enter_context(tc.tile_pool(name="feat", bufs=1))
            wxyp = ctx.enter_context(tc.tile_pool(name="wxy", bufs=2))
            wyrp = ctx.enter_context(tc.tile_pool(name="wyr", bufs=2))
            wp = ctx.enter_context(tc.tile_pool(name="w", bufs=8))
            recp = ctx.enter_context(tc.tile_pool(name="rec", bufs=2))
            outp = ctx.enter_context(tc.tile_pool(name="outsb", bufs=4))
            psprep = ctx.enter_context(
                tc.tile_pool(name="psprep", bufs=2, space="PSUM"))
            pse = ctx.enter_context(
                tc.tile_pool(name="pse", bufs=2, space="PSUM"))
            psmisc = ctx.enter_context(
                tc.tile_pool(name="psmisc", bufs=2, space="PSUM"))

            pk_sb = const.tile([KPREP, 96 + N], bf16)
            oner_sb = const.tile([1, 128], f32r)
            nc.vector.memset(oner_sb[:].bitcast(mybir.dt.float32), 1.0)
            # warm up the ACT Exp function table before the first real Exp
            warm = const.tile([1, 8], f32)
            warm2 = const.tile([1, 8], f32)
            nc.vector.memset(warm[:], 0.0)
            nc.scalar.activation(warm2[:], warm[:],
                                 mybir.ActivationFunctionType.Exp)
            # HWDGE is one serial resource with ~625ns fixed cost per
            # DMA: lead with the fused crdk+posk-head transfer the first
            # prep groups need, and ramp feat chunk sizes to match the
            # paired phase-1 consumption pace
            feat_sb = featp.tile([128, NB * E], bf16)

            def feat_dma(eng, at, fch):
                src = feat_d[at * 128:(at + fch) * 128, :] \
                    .rearrange("(nb p) e -> p nb e", nb=fch)
                dst = feat_sb[:, at * E:(at + fch) * E] \
                    .rearrange("p (nb e) -> p nb e", nb=fch)
                eng.dma_start(dst, src)

            # one fused DMA carries crdk + the posk head (everything the
            # first prep groups need) -- single HWDGE slot + sem hop
            nc.sync.dma_start(pk_sb[:, 0:1120], pk_d[:, 0:1120])
            feat_dma(nc.sync, 0, 2)
            nc.sync.dma_start(pk_sb[:, 1120:], pk_d[:, 1120:])
            feat_dma(nc.sync, 2, 2)
            at = 4
            for fch in (4, 6, 6, 6, 6):
                feat_dma(nc.sync, at, fch)
                at += fch

            pools = dict(wp=wp, recp=recp, outp=outp, wxyp=wxyp,
                         wyrp=wyrp, psprep=psprep, pse=pse, psmisc=psmisc,
                         feat_sb=feat_sb, pk_sb=pk_sb,
                         oner_sb=oner_sb, out_d=out_d)
            if reps == 1:
                _emit(nc, pools)
            else:
                with tc.For_i(0, reps, 1):
                    _emit(nc, pools)

    nc.compile()
    _CACHE[reps] = nc
    return nc


def _emit(nc, pools):
    f32 = mybir.dt.float32
    f32r = mybir.dt.float32r
    bf16 = mybir.dt.bfloat16
    Exp = mybir.ActivationFunctionType.Exp
    Copy = mybir.ActivationFunctionType.Copy
    wp, recp, outp = pools["wp"], pools["recp"], pools["outp"]
    psprep, pse, psmisc = pools["psprep"], pools["pse"], pools["psmisc"]
    feat_sb, pk_sb = pools["feat_sb"], pools["pk_sb"]
    oner_sb, out_d = pools["oner_sb"], pools["out_d"]
    bf16_ = mybir.dt.bfloat16
    wxy_sb = pools["wxyp"].tile([128, NB * 96], bf16_)
    wyr_sb = pools["wyrp"].tile([128, NB * 512], bf16_)

    # progressive group sizes: the first wxy ACT fires after a single
    # 96-col matmul, shortening the chain to the first W tile
    sizes = [1, 2, 5, 5, 5, 5, 5, 4]
    groups, at = [], 0
    for s in sizes:
        groups.append(list(range(at, at + s)))
        at += s

    # --- prep: exponent matmuls + group ACT Exp -> wxy_sb.
    #     Groups g0/g1 are emitted up front (they gate the first W tiles);
    #     later groups interleave into the phase-1 matmul stream so the PE
    #     never idles on the psum ring. ---
    def emit_prep(gi, wyr=True):
        grp = groups[gi]
        ps = psprep.tile([128, 96 * GRP], f32)
        for i, nb in enumerate(grp):
            nc.tensor.matmul(ps[:, 96 * i:96 * (i + 1)],
                             pk_sb[:, 96 + nb * 128:96 + (nb + 1) * 128],
                             pk_sb[:, 0:96],
                             start=True, stop=True)
        nc.scalar.activation(wxy_sb[:, grp[0] * 96:(grp[-1] + 1) * 96],
                             ps[:, 0:96 * len(grp)], Exp, scale=SCALE)
        if wyr:
            for nb in grp:
                emit_wyr(nb)

    # --- den: [32gx, 64gy] psum accumulation, emitted densely into the
    #     early phase-1 stream as the wxy groups become available ---
    den = psmisc.tile([GXC, 64], f32, name="psmisc", bufs=2)

    def emit_den(nb):
        nc.tensor.matmul(den[:],
                         wxy_sb[:, nb * 96:nb * 96 + GXC],
                         wxy_sb[:, nb * 96 + GXC:(nb + 1) * 96],
                         start=(nb == 0), stop=(nb == NB - 1))

    # --- wyR: replicate wy across the gx octet, interleaved col=gy*8+k.
    #     Split across ACT (even blocks) and Pool (odd) so neither paces
    #     the paired-phase feature matmuls. ---
    def emit_wyr(nb):
        src = wxy_sb[:, nb * 96 + GXC:(nb + 1) * 96] \
            .unsqueeze(2).broadcast_to([128, 64, 8])
        dst = wyr_sb[:, nb * 512:(nb + 1) * 512] \
            .rearrange("p (gy k) -> p gy k", k=8)
        if nb % 2 == 0:
            nc.scalar.activation(dst, src, Copy)
        else:
            nc.gpsimd.tensor_copy(dst, src)

    emit_prep(0)
    emit_prep(1)

    rec2d = recp.tile([GXC, 64], f32r)
    recr = recp.tile([1, HALF], f32r)

    def emit_norm(st, tail=False):
        j, e0, e1 = st
        bc = bcs.pop(j)
        # DVE can read only one PSUM operand; stage bc in SBUF via the
        # (otherwise idle) ACT engine
        bc_sb = outp.tile([128, 512], f32, name="bcsb", bufs=2)
        nc.scalar.activation(bc_sb[:], bc[:], Copy)
        # o01[p, h, col]: e-half h, output col in natural k*64+gy order;
        # the de-interleave (col gy*8+k -> k*64+gy) rides the f32 norm
        # mult's output AP for free, and one DMA ships both e-halves
        o01 = outp.tile([128, 2, 512], f32)
        bv = bc_sb[:].rearrange("p (gy k) -> p gy k", k=8)
        ev0 = e0[:].rearrange("p (gy k) -> p gy k", k=8)
        ev1 = e1[:].rearrange("p (gy k) -> p gy k", k=8)

        def ov(h, klo, khi):
            return o01[:, h, klo * 64:khi * 64] \
                .rearrange("p (k gy) -> p gy k", gy=64, k=khi - klo)

        if not tail:
            nc.vector.tensor_mul(ov(0, 0, 8), ev0, bv)
            nc.vector.tensor_mul(ov(1, 0, 8), ev1, bv)
            nc.sync.dma_start(out_d[:, :, j * 512:(j + 1) * 512], o01[:])
        else:
            # last tile: halves pipelined with two DMAs on separate
            # HWDGE queues, so the drain tail is short
            nc.vector.tensor_mul(ov(0, 0, 4), ev0[:, :, 0:4], bv[:, :, 0:4])
            nc.vector.tensor_mul(ov(1, 0, 4), ev1[:, :, 0:4], bv[:, :, 0:4])
            nc.sync.dma_start(out_d[:, :, j * 512:j * 512 + 256],
                              o01[:, :, 0:256])
            nc.vector.tensor_mul(ov(0, 4, 6), ev0[:, :, 4:6], bv[:, :, 4:6])
            nc.vector.tensor_mul(ov(1, 4, 6), ev1[:, :, 4:6], bv[:, :, 4:6])
            nc.scalar.dma_start(out_d[:, :, j * 512 + 256:j * 512 + 384],
                                o01[:, :, 256:384])
            nc.vector.tensor_mul(ov(0, 6, 8), ev0[:, :, 6:8], bv[:, :, 6:8])
            nc.vector.tensor_mul(ov(1, 6, 8), ev1[:, :, 6:8], bv[:, :, 6:8])
            nc.sync.dma_start(out_d[:, :, j * 512 + 384:(j + 1) * 512],
                                o01[:, :, 384:512])

    def emit_bc(j):
        bc = psmisc.tile([128, 512], f32, name="psmisc", bufs=2)
        nc.tensor.matmul(bc[:], oner_sb[:],
                         recr[:, j * 512:(j + 1) * 512]
                         .rearrange("o (k gy) -> o gy k", k=8),
                         start=True, stop=True)
        bcs[j] = bc

    ename = [("ea0", "ea1"), ("eb0", "eb1")]
    etiles = {}

    def alloc_e(j):
        n0, n1 = ename[j % 2]
        etiles[j] = (pse.tile([128, 512], f32, name=n0, bufs=1),
                     pse.tile([128, 512], f32, name=n1, bufs=1))

    def build_w(j, nb, direct=False, pool_ok=False):
        w = wp.tile([128, 512], bf16)
        w_v = w[:].rearrange("p (gy k) -> p gy k", k=8)
        wx_b = wxy_sb[:, nb * 96 + j * 8:nb * 96 + j * 8 + 8] \
            .unsqueeze(1).broadcast_to([128, 64, 8])
        if direct:
            # stride-0 wy read (1x DVE) skips the wyR chain hop; used for
            # the first blocks while the pipeline warms up
            wyr_v = wxy_sb[:, nb * 96 + GXC:(nb + 1) * 96] \
                .unsqueeze(2).broadcast_to([128, 64, 8])
        else:
            wyr_v = wyr_sb[:, nb * 512:(nb + 1) * 512] \
                .rearrange("p (gy k) -> p gy k", k=8)
        if pool_ok and nb % 4 == 3:
            # hedge: every 4th tile on the (idle) Pool engine -- free in
            # the cost model, and halves the gap if real-hardware DVE
            # runs these below the modeled 2x rate
            nc.gpsimd.tensor_mul(w_v, wx_b, wyr_v)
        else:
            nc.vector.tensor_mul(w_v, wx_b, wyr_v)
        return w

    def mm(j, nb, w):
        e0, e1 = etiles[j]
        st, sp = (nb == 0), (nb == NB - 1)
        nc.tensor.matmul(e0[:], feat_sb[:, nb * E:nb * E + 128],
                         w[:], start=st, stop=sp)
        nc.tensor.matmul(e1[:], feat_sb[:, nb * E + 128:(nb + 1) * E],
                         w[:], start=st, stop=sp)

    def emit_recip():
        with nc.allow_low_precision(reason="f32r bit-identical"):
            nc.vector.reciprocal(rec2d[:], den[:])
        for jj in range(NJ):
            nc.scalar.dma_start(
                recr[:, jj * 512:(jj + 1) * 512]
                .rearrange("o (k gy) -> o k gy", gy=64, k=8),
                rec2d[8 * jj:8 * jj + 8, :])

    bcs = {}
    # phase 1: j0+j1 paired over nb 0..15 -- halves the supply pressure
    # (wyR, feat DMA) while those streams are still warming up
    alloc_e(0)
    alloc_e(1)
    den_q = 0
    for i in range(16):
        w0 = build_w(0, i, direct=(i < 2))
        w1 = build_w(1, i, direct=(i < 2))
        if 2 <= i < 8:
            emit_prep(i)          # groups g2..g7 interleave here
        mm(0, i, w0)
        mm(1, i, w1)
        if i >= 2:
            lim = min(NB, 3 * (i - 1))
            while den_q < lim:
                emit_den(den_q)
                den_q += 1
        if i == 13:
            emit_recip()
    while den_q < NB:
        emit_den(den_q)
        den_q += 1
    # phase 2: finish j0 (supply streams are ahead now)
    for nb in range(16, NB):
        w = build_w(0, nb, pool_ok=True)
        if nb == 18:
            emit_bc(0)
        mm(0, nb, w)
    # phase 3: finish j1; norm j0 slots in once its psum stops
    for nb in range(16, NB):
        w = build_w(1, nb, pool_ok=True)
        if nb == 18:
            emit_bc(1)
        if nb == 20:
            emit_norm((0,) + etiles[0])
        mm(1, nb, w)
    # phases 4/5: j2 and j3 full-rate; e-psum names reused after norms
    for j in (2, 3):
        alloc_e(j)
        for nb in range(NB):
            w = build_w(j, nb, pool_ok=True)
            if nb == 2 and j == 3:
                emit_bc(2)
            if nb == 4:
                emit_norm((j - 1,) + etiles[j - 1])
            if nb == 20 and j == 3:
                emit_bc(3)
            mm(j, nb, w)
    emit_norm((3,) + etiles[3], tail=True)


def _host_prep(neuron_features, positions):
    """Per-core input maps. Core c: batch c//2, grid half c%2."""
    lin = np.linspace(0.0, 1.0, G).astype(np.float64)

    crdk_halves = []
    for h in range(2):
        cx = lin[h * GXC:(h + 1) * GXC]
        cy = lin
        a1, a2 = _split2(2.0 * cx)
        b1, b2 = _split2(2.0 * cy)
        ncx1, ncx2 = _split2(-cx * cx)
        ncy1, ncy2 = _split2(-cy * cy)
        z32 = np.zeros(GXC, dtype=BF16)
        z64 = np.zeros(64, dtype=BF16)
        o32 = np.ones(GXC, dtype=BF16)
        o64 = np.ones(64, dtype=BF16)
        xcols = np.stack([a1, a2, a1, z32, z32, z32,
                          o32, o32, z32, z32, ncx1, ncx2], axis=0)
        ycols = np.stack([z64, z64, z64, b1, b2, b1,
                          z64, z64, o64, o64, ncy1, ncy2], axis=0)
        crdk_halves.append(
            np.concatenate([xcols, ycols], axis=1).astype(BF16))

    posks, feats = [], []
    for b in range(B):
        x = positions[b, :, 0].astype(np.float64)
        y = positions[b, :, 1].astype(np.float64)
        x1, x2 = _split2(x)
        y1, y2 = _split2(y)
        nx1, nx2 = _split2(-(x * x))
        ny1, ny2 = _split2(-(y * y))
        one = np.ones(N, dtype=BF16)
        posks.append(np.stack([x1, x1, x2, y1, y1, y2,
                               nx1, nx2, ny1, ny2, one, one],
                              axis=0).astype(BF16))
        feats.append(np.ascontiguousarray(
            neuron_features[b].astype(BF16)))

    in_maps = []
    for c in range(N_CORES):
        b, h = divmod(c, 2)
        in_maps.append({
            "feat": feats[b],
            "pk": np.concatenate([crdk_halves[h], posks[b]], axis=1),
        })
    return in_maps


def kernel(neuron_features, positions):
    global LAST_EXEC_NS, LAST_RESULTS
    nf = np.ascontiguousarray(np.asarray(neuron_features, dtype=np.float32))
    pos = np.ascontiguousarray(np.asarray(positions, dtype=np.float32))
    nc = _build()
    in_maps = _host_prep(nf, pos)
    trace = bool(int(os.environ.get("KERNEL_TRACE", "0")))
    res = bass_utils.run_bass_kernel_spmd(nc, in_maps,
                                          core_ids=list(range(N_CORES)),
                                          trace=trace)
    LAST_RESULTS = res
    LAST_EXEC_NS = getattr(res, "exec_time_ns", None)
    full = np.empty((B, E, P), np.float32)
    for c in range(N_CORES):
        b, h = divmod(c, 2)
        o = res.results[c]["out"]  # [128, 2, HALF]
        full[b, :, h * HALF:(h + 1) * HALF] = \
            o.transpose(1, 0, 2).reshape(E, HALF)
    return full.reshape(B, E, G, G)
